# revision 32
# baseline (speedup 1.0000x reference)
"""Trainium2 Bass kernel for nn_G_HGNN_layer_38448547234609.

HGNN layer: knn-hypergraph construction (top-11 nearest of 8192 nodes) +
static local-window hyperedges, G = Dv^-1/2 H De^-1 H^T Dv^-1/2 message
passing, linear -> G @ y -> batchnorm(train) -> relu -> residual.

Never materializes G: z = dv2 * (Hfull @ (1/DE * (Hfull^T @ (dv2*y)))).

Sharding: core c owns sample c (1024 nodes = 8 row-tiles of 128).
v2 design (vs baseline): knn masks kept SBUF-resident in fp8 (64KB/part,
no DRAM spill), u accumulated in two PSUM passes re-reading SBUF masks,
two pipelined AllReduce halves, masks transposed IN PLACE on PE (fp8,
batched 16-chunk PSUM evictions split Act/DVE) during the collectives,
z matmuls with fp8 lhsT x bf16 rhs.
"""

import numpy as np
import ml_dtypes

import concourse.bass as bass
import concourse.bacc as bacc
import concourse.mybir as mybir
import concourse.tile as tile
from concourse import bass_utils

AF = mybir.ActivationFunctionType
ALU = mybir.AluOpType
F32 = mybir.dt.float32
BF16 = mybir.dt.bfloat16
FP8 = mybir.dt.float8e4

NODE, K, KER, STR = 32, 10, 5, 2
B, C = 8, 64
N = NODE * NODE            # 1024 nodes/sample
BN = B * N                 # 8192
OUT_ = (NODE - KER) // STR + 1
E = OUT_ * OUT_            # 196 local hyperedges/sample
NCORE = 8
NT = 8                     # 128-row tiles per core
JC = 64                    # 128-col j-chunks
BN_EPS = 1e-5
BIG = 1e30

_CACHE = {}
SIM_NO_CC = False  # replace collectives with DMA copies (for TimelineSim)
DEBUG = False      # add intermediate dumps (dbg_* outputs)


def _local_incidence():
    idx = np.arange(N).reshape(NODE, NODE)
    H_local = np.zeros((N, E), np.float32)
    e = 0
    for i in range(0, NODE - KER + 1, STR):
        for j in range(0, NODE - KER + 1, STR):
            H_local[idx[i:i + KER, j:j + KER].ravel(), e] = 1.0
            e += 1
    return H_local


def _u_off(slot):
    # 32 PSUM accumulators of width 65 packed 7-per-512-f32-bank (5 banks)
    return (slot // 7) * 512 + (slot % 7) * 65


def _u_sb_off(jc):
    # u_sb mirrors the PSUM bank packing so drains are 5 wide copies per half
    h = 0 if jc < 32 else 32 * 65
    s = jc % 32
    return h + (s // 7) * 455 + (s % 7) * 65


_BANKW = [455, 455, 455, 455, 260]  # 7,7,7,7,4 slots of 65


def _build():
    nc = bacc.Bacc(num_devices=NCORE)

    bz = nc.dram_tensor("bz", [65, BN], F32, kind="ExternalInput")
    acore = nc.dram_tensor("acore", [65, N], F32, kind="ExternalInput")
    wb = nc.dram_tensor("wb", [65, C], F32, kind="ExternalInput")
    dv2t = nc.dram_tensor("dv2t", [128, NT], F32, kind="ExternalInput")
    hloc = nc.dram_tensor("hloc", [128, NT * E], BF16, kind="ExternalInput")
    hloct = nc.dram_tensor("hloct", [98, 2 * NT * 128], BF16, kind="ExternalInput")
    ident = nc.dram_tensor("ident", [128, 128], FP8, kind="ExternalInput")
    gamma = nc.dram_tensor("gamma", [1, C], F32, kind="ExternalInput")
    beta = nc.dram_tensor("beta", [1, C], F32, kind="ExternalInput")
    xres = nc.dram_tensor("xres", [128, NT * C], F32, kind="ExternalInput")
    out = nc.dram_tensor("out", [N, C], F32, kind="ExternalOutput")
    if DEBUG:
        dbg_u = nc.dram_tensor("dbg_u", [128, JC * 65], BF16, kind="ExternalOutput")
        dbg_ur = nc.dram_tensor("dbg_ur", [128, JC * 65], BF16, kind="ExternalOutput")
        dbg_v = nc.dram_tensor("dbg_v", [128, JC * C], BF16, kind="ExternalOutput")
        dbg_z = nc.dram_tensor("dbg_z", [128, NT * C], F32, kind="ExternalOutput")
        dbg_m = nc.dram_tensor("dbg_m", [128, NT * BN], FP8, kind="ExternalOutput")
        dbg_st = nc.dram_tensor("dbg_st", [1, 128], F32, kind="ExternalOutput")
        dbg_mt = nc.dram_tensor("dbg_mt", [128, NT * BN], FP8, kind="ExternalOutput")

    with tile.TileContext(nc) as tc:
        with (
            tc.tile_pool(name="const", bufs=1) as cp,
            tc.tile_pool(name="small", bufs=4) as sp,
            tc.tile_pool(name="persist", bufs=1) as pp,
            tc.tile_pool(name="dram", bufs=1, space="DRAM") as dr,
        ):
            # ---- const loads (bz split so P1's first chunks start early) ----
            bz_sb = cp.tile([65, BN], F32, tag="bz")
            for q in range(4):
                nc.sync.dma_start(bz_sb[:, q * 2048:(q + 1) * 2048],
                                  bz[:, q * 2048:(q + 1) * 2048])
            ac_sb = cp.tile([65, N], F32, tag="ac")
            nc.sync.dma_start(ac_sb[:], acore[:])
            wb_sb = cp.tile([65, C], F32, tag="wb")
            nc.sync.dma_start(wb_sb[:], wb[:])
            dv2_sb = cp.tile([128, NT], F32, tag="dv2")
            nc.sync.dma_start(dv2_sb[:], dv2t[:])
            hloc_sb = cp.tile([128, NT * E], BF16, tag="hloc")
            nc.sync.dma_start(hloc_sb[:], hloc[:])
            hloct_sb = cp.tile([98, 2 * NT * 128], BF16, tag="hloct")
            nc.sync.dma_start(hloct_sb[:], hloct[:])
            id_sb = cp.tile([128, 128], FP8, tag="ident")
            nc.sync.dma_start(id_sb[:], ident[:])
            gam_sb = cp.tile([1, C], F32, tag="gamma")
            nc.sync.dma_start(gam_sb[:], gamma[:])
            bet_sb = cp.tile([1, C], F32, tag="beta")
            nc.sync.dma_start(bet_sb[:], beta[:])
            xr_sb = cp.tile([128, NT * C], F32, tag="xres")
            nc.sync.dma_start(xr_sb[:], xres[:])

            ones_sb = pp.tile([128, 1], F32, tag="ones")
            nc.vector.memset(ones_sb[:], 1.0)
            m_aug = pp.tile([128, NT * 65], BF16, tag="maug")
            mask_sb = pp.tile([128, NT * BN], FP8, tag="mask")
            d_sb = pp.tile([128, BN], F32, tag="dsb")
            u_sb = pp.tile([128, JC * 65], BF16, tag="usb")
            ur_sb = pp.tile([128, JC * 65], BF16, tag="ursb")
            v_sb = pp.tile([128, JC * C], BF16, tag="vsb")
            vloc_sb = pp.tile([98, 2 * C], BF16, tag="vloc")
            z_sb = pp.tile([128, NT * C], F32, tag="zsb")
            zsq_sb = pp.tile([128, C], F32, tag="zsq")

            def slot(it, jc):
                return mask_sb[:, it * BN + jc * 128:it * BN + (jc + 1) * 128]

            # ---- P0: y = x W^T + b ; m = dv2*y (bf16), ones col ----
            nc.vector.memset(m_aug[:], 1.0)  # ones cols; m cols overwritten below
            with tc.tile_pool(name="py", bufs=2, space="PSUM") as pyp:
                for it in range(NT):
                    y_ps = pyp.tile([128, C], F32, tag="y")
                    nc.tensor.matmul(y_ps[:], lhsT=ac_sb[:, it * 128:(it + 1) * 128],
                                     rhs=wb_sb[:], start=True, stop=True)
                    nc.scalar.activation(m_aug[:, it * 65:it * 65 + C], y_ps[:],
                                         AF.Copy, bias=0.0, scale=dv2_sb[:, it:it + 1])

            # ---- P1: distances, exact top-11 threshold, fp8 mask, u1 ----
            with (
                tc.tile_pool(name="pd", bufs=3, space="PSUM") as pdp,
                tc.tile_pool(name="pu", bufs=1, space="PSUM") as pup,
            ):
                u_ps = pup.tile([128, 5 * 512], F32, tag="u")
                nc.vector.memset(u_ps[:], 0.0)
                for it in range(NT):
                    for nck in range(16):
                        d_ps = pdp.tile([128, 512], F32, tag="dch")
                        nc.tensor.matmul(d_ps[:],
                                         lhsT=ac_sb[:, it * 128:(it + 1) * 128],
                                         rhs=bz_sb[:, nck * 512:(nck + 1) * 512],
                                         start=True, stop=True)
                        nc.scalar.copy(d_sb[:, nck * 512:(nck + 1) * 512], d_ps[:])
                    # top-8 per 512-wide segment -> 128 candidates/row.
                    # For this problem no row has >8 of its top-11 in one
                    # segment (max observed 6), so candidates contain the
                    # exact global top-11; T11 = 11th largest candidate.
                    cand = sp.tile([128, 128], F32, tag="cand")
                    for sg in range(16):
                        nc.vector.max(cand[:, sg * 8:(sg + 1) * 8],
                                      d_sb[:, sg * 512:(sg + 1) * 512])
                    c8a = sp.tile([128, 8], F32, tag="v8")
                    nc.vector.max(c8a[:], cand[:])
                    nc.vector.match_replace(cand[:], c8a[:], cand[:], -BIG)
                    c8b = sp.tile([128, 8], F32, tag="v8")
                    nc.vector.max(c8b[:], cand[:])
                    # mask in 4 quarters (finer WAR granularity on d_sb)
                    for q in range(4):
                        nc.vector.tensor_scalar(
                            mask_sb[:, it * BN + q * 2048:it * BN + (q + 1) * 2048],
                            d_sb[:, q * 2048:(q + 1) * 2048],
                            c8b[:, 2:3], None, ALU.is_ge)
                    for jc in range(32):
                        o = _u_off(jc)
                        nc.tensor.matmul(u_ps[:, o:o + 65],
                                         lhsT=slot(it, jc),
                                         rhs=m_aug[:, it * 65:(it + 1) * 65],
                                         start=False, stop=(it == NT - 1),
                                         skip_group_check=True)
                # drain u1: one wide copy per PSUM bank
                for k in range(5):
                    w = _BANKW[k]
                    nc.scalar.copy(u_sb[:, k * 455:k * 455 + w],
                                   u_ps[:, k * 512:k * 512 + w])

                # ---- P2: u2 from SBUF-resident masks (jc-major) ----
                nc.vector.memset(u_ps[:], 0.0)
                for jc in range(32, 64):
                    o = _u_off(jc - 32)
                    for it in range(NT):
                        nc.tensor.matmul(u_ps[:, o:o + 65],
                                         lhsT=slot(it, jc),
                                         rhs=m_aug[:, it * 65:(it + 1) * 65],
                                         start=False, stop=(it == NT - 1),
                                         skip_group_check=True)
                for k in range(5):
                    w = _BANKW[k]
                    nc.scalar.copy(u_sb[:, 2080 + k * 455:2080 + k * 455 + w],
                                   u_ps[:, k * 512:k * 512 + w])
                if DEBUG:
                    nc.sync.dma_start(dbg_u[:], u_sb[:])
                    nc.sync.dma_start(dbg_m[:], mask_sb[:])
                # single bf16 AllReduce of u (1.06MB)
                cc_in1 = dr.tile([128, JC * 65], BF16, tag="ccin1")
                cc_out1 = dr.tile([128, JC * 65], BF16, tag="ccout1", addr_space="Shared")
                nc.sync.dma_start(cc_in1[:], u_sb[:])
                if SIM_NO_CC:
                    nc.sync.dma_start(cc_out1[:], cc_in1[:])
                else:
                    nc.gpsimd.collective_compute(
                        "AllReduce", ALU.add, replica_groups=[list(range(NCORE))],
                        ins=[cc_in1.opt()], outs=[cc_out1.opt()])

            # ---- local hyperedge t (independent of collectives) ----
            with tc.tile_pool(name="ptl", bufs=2, space="PSUM") as ptlp:
                tl_ps = [ptlp.tile([98, C], F32, tag=f"tl{ec}", name=f"tl_ps{ec}")
                         for ec in range(2)]
                for it in range(NT):
                    for ec in range(2):
                        nc.tensor.matmul(tl_ps[ec][:],
                                         lhsT=hloc_sb[:, it * E + ec * 98:it * E + ec * 98 + 98],
                                         rhs=m_aug[:, it * 65:it * 65 + C],
                                         start=(it == 0), stop=(it == NT - 1))
                for ec in range(2):
                    nc.scalar.activation(vloc_sb[:, ec * C:(ec + 1) * C], tl_ps[ec][:],
                                         AF.Copy, bias=0.0, scale=1.0 / 25.0)

            # ---- in-place mask transposes (fill PE during collectives) ----
            # Regular matmul vs identity (NOT is_transpose: the tile framework
            # serializes is_transpose instructions against collectives, which
            # would push this whole phase out of the collective window).
            # 32 groups of 16 chunks; f32 PSUM (4 banks/group, 2 bufs).
            with tc.tile_pool(name="ptr", bufs=2, space="PSUM") as ptrp:
                groups = [(it, h) for h in range(4) for it in range(NT)]
                for gi, (it, h) in enumerate(groups):
                    t_ps = ptrp.tile([128, 2048], F32, tag="tp")
                    for k in range(16):
                        jc = h * 16 + k
                        nc.tensor.matmul(t_ps[:, k * 128:(k + 1) * 128],
                                         lhsT=slot(it, jc), rhs=id_sb[:],
                                         start=True, stop=True)
                    dst = mask_sb[:, it * BN + h * 2048:it * BN + (h + 1) * 2048]
                    nc.scalar.copy(dst, t_ps[:])

            # ---- P4: readback + v = t / DE ----
            nc.sync.dma_start(ur_sb[:], cc_out1[:])
            # one reciprocal of the whole buffer (non-DE lanes unused garbage)
            urec = pp.tile([128, JC * 65], F32, tag="urec")
            nc.vector.reciprocal(urec[:], ur_sb[:])
            for jc in range(JC):
                o = _u_sb_off(jc)
                nc.vector.tensor_scalar(v_sb[:, jc * C:(jc + 1) * C],
                                        ur_sb[:, o:o + C],
                                        urec[:, o + C:o + C + 1], None, ALU.mult)

            if DEBUG:
                nc.sync.dma_start(dbg_ur[:], ur_sb[:])
                nc.sync.dma_start(dbg_v[:], v_sb[:])
                nc.sync.dma_start(dbg_mt[:], mask_sb[:])

            # ---- P5: z = H v + local (all 8 z accumulators in one PSUM bank) ----
            with (
                tc.tile_pool(name="pz", bufs=1, space="PSUM") as pzp,
                tc.tile_pool(name="pst", bufs=1, space="PSUM") as pstp,
            ):
                z_ps = pzp.tile([128, NT * C], F32, tag="z")
                st_ps = pstp.tile([1, 128], F32, tag="st")
                nc.vector.memset(z_ps[:], 0.0)
                nc.vector.memset(st_ps[:], 0.0)
                for g in range(2):
                    for it in range(NT):
                        for jc in range(g * 32, (g + 1) * 32):
                            nc.tensor.matmul(z_ps[:, it * C:(it + 1) * C],
                                             lhsT=slot(it, jc),
                                             rhs=v_sb[:, jc * C:(jc + 1) * C],
                                             start=False, stop=False,
                                             skip_group_check=True)
                for it in range(NT):
                    for ec in range(2):
                        nc.tensor.matmul(z_ps[:, it * C:(it + 1) * C],
                                         lhsT=hloct_sb[:, (ec * NT + it) * 128:(ec * NT + it + 1) * 128],
                                         rhs=vloc_sb[:, ec * C:(ec + 1) * C],
                                         start=False, stop=(ec == 1),
                                         skip_group_check=True)
                    # z scaled by dv2 on copy out
                    nc.scalar.activation(z_sb[:, it * C:(it + 1) * C],
                                         z_ps[:, it * C:(it + 1) * C],
                                         AF.Copy, bias=0.0, scale=dv2_sb[:, it:it + 1])
                    nc.vector.tensor_tensor(zsq_sb[:], z_sb[:, it * C:(it + 1) * C],
                                            z_sb[:, it * C:(it + 1) * C], ALU.mult)
                    nc.tensor.matmul(st_ps[0:1, 0:C], lhsT=ones_sb[:, 0:1],
                                     rhs=z_sb[:, it * C:(it + 1) * C],
                                     start=False, stop=(it == NT - 1),
                                     skip_group_check=True)
                    nc.tensor.matmul(st_ps[0:1, C:2 * C], lhsT=ones_sb[:, 0:1],
                                     rhs=zsq_sb[:],
                                     start=False, stop=(it == NT - 1),
                                     skip_group_check=True)
                st_sb = sp.tile([1, 128], F32, tag="stsb")
                nc.scalar.copy(st_sb[:], st_ps[:])
            if DEBUG:
                nc.sync.dma_start(dbg_z[:], z_sb[:])
                nc.sync.dma_start(dbg_st[:], st_sb[:])

            # ---- P6: AllGather stats + local partition-sum, BN coefficients ----
            st_in = dr.tile([1, 128], F32, tag="stin")
            st_out = dr.tile([NCORE, 128], F32, tag="stout", addr_space="Shared")
            nc.sync.dma_start(st_in[:], st_sb[:])
            if SIM_NO_CC:
                for c_ in range(NCORE):
                    nc.sync.dma_start(st_out[c_:c_ + 1, :], st_in[:])
            else:
                nc.gpsimd.collective_compute(
                    "AllGather", ALU.bypass, replica_groups=[list(range(NCORE))],
                    ins=[st_in.opt()], outs=[st_out.opt()])
            stg8 = sp.tile([NCORE, 128], F32, tag="stg8")
            nc.sync.dma_start(stg8[:], st_out[:])
            stg = sp.tile([1, 128], F32, tag="stg")
            with tc.tile_pool(name="pss", bufs=1, space="PSUM") as pssp:
                ss_ps = pssp.tile([1, 128], F32, tag="ss")
                nc.tensor.matmul(ss_ps[:], lhsT=ones_sb[0:NCORE, 0:1], rhs=stg8[:],
                                 start=True, stop=True)
                nc.scalar.copy(stg[:], ss_ps[:])

            mu = sp.tile([1, C], F32, tag="mu")
            nc.vector.tensor_scalar(mu[:], stg[0:1, 0:C], 1.0 / BN, None, ALU.mult)
            ex2 = sp.tile([1, C], F32, tag="ex2")
            nc.vector.tensor_scalar(ex2[:], stg[0:1, C:2 * C], 1.0 / BN, None, ALU.mult)
            musq = sp.tile([1, C], F32, tag="musq")
            nc.vector.tensor_tensor(musq[:], mu[:], mu[:], ALU.mult)
            var = sp.tile([1, C], F32, tag="var")
            nc.vector.tensor_tensor(var[:], ex2[:], musq[:], ALU.subtract)
            eps_t = sp.tile([1, 1], F32, tag="eps")
            nc.vector.memset(eps_t[:], BN_EPS)
            sd = sp.tile([1, C], F32, tag="sd")
            nc.scalar.activation(sd[:], var[:], AF.Sqrt, bias=eps_t[0:1, 0:1], scale=1.0)
            inv = sp.tile([1, C], F32, tag="inv")
            nc.vector.reciprocal(inv[:], sd[:])
            srow = sp.tile([1, C], F32, tag="srow")
            nc.vector.tensor_tensor(srow[:], gam_sb[:], inv[:], ALU.mult)
            msr = sp.tile([1, C], F32, tag="msr")
            nc.vector.tensor_tensor(msr[:], mu[:], srow[:], ALU.mult)
            trow = sp.tile([1, C], F32, tag="trow")
            nc.vector.tensor_tensor(trow[:], bet_sb[:], msr[:], ALU.subtract)
            s_b = pp.tile([128, C], F32, tag="sb_b")
            nc.gpsimd.partition_broadcast(s_b[:], srow[:])
            t_b = pp.tile([128, C], F32, tag="tb_b")
            nc.gpsimd.partition_broadcast(t_b[:], trow[:])

            # ---- P7: out = relu(z*s + t) + x ----
            for it in range(NT):
                tmp = sp.tile([128, C], F32, tag="tmp")
                nc.vector.tensor_tensor(tmp[:], z_sb[:, it * C:(it + 1) * C], s_b[:], ALU.mult)
                nc.vector.tensor_tensor(tmp[:], tmp[:], t_b[:], ALU.add)
                nc.scalar.activation(tmp[:], tmp[:], AF.Relu, bias=0.0, scale=1.0)
                ot = sp.tile([128, C], F32, tag="ot")
                nc.vector.tensor_tensor(ot[:], tmp[:], xr_sb[:, it * C:(it + 1) * C], ALU.add)
                nc.sync.dma_start(out[it * 128:(it + 1) * 128, :], ot[:])

    nc.compile()
    return nc


def _host_inputs(x, W_conv, b_conv, gamma, beta):
    xm = np.ascontiguousarray(x.reshape(BN, C).astype(np.float32))
    xT = np.ascontiguousarray(xm.T)
    sq = (xm * xm).sum(1).astype(np.float32)

    bz = np.concatenate([2.0 * xT, -sq[None, :]], 0).astype(np.float32)
    wbm = np.concatenate([W_conv.T.astype(np.float32), b_conv[None, :].astype(np.float32)], 0)

    H_local = _local_incidence()
    cover = H_local.sum(1)
    dv2 = ((K + 1 + cover) ** -0.5).astype(np.float32)
    dv2t = dv2.reshape(NT, 128).T.copy()  # [128, NT]

    hloc = np.zeros((128, NT * E), np.float32)
    for it in range(NT):
        hloc[:, it * E:(it + 1) * E] = H_local[it * 128:(it + 1) * 128, :]
    hloct = np.zeros((98, 2 * NT * 128), np.float32)
    for ec in range(2):
        for it in range(NT):
            blk = H_local[it * 128:(it + 1) * 128, ec * 98:ec * 98 + 98].T
            hloct[:, (ec * NT + it) * 128:(ec * NT + it + 1) * 128] = blk

    ident = np.eye(128, dtype=np.float32)
    bf = ml_dtypes.bfloat16
    f8 = np.dtype(mybir.dt.np(FP8))
    common = {
        "bz": bz,
        "wb": wbm,
        "dv2t": dv2t,
        "hloc": hloc.astype(bf),
        "hloct": hloct.astype(bf),
        "ident": ident.astype(f8),
        "gamma": np.ascontiguousarray(gamma.astype(np.float32)[None, :]),
        "beta": np.ascontiguousarray(beta.astype(np.float32)[None, :]),
    }
    in_maps = []
    for c in range(NCORE):
        acore = np.concatenate(
            [xT[:, c * N:(c + 1) * N], np.ones((1, N), np.float32)], 0)
        xr = np.zeros((128, NT * C), np.float32)
        for it in range(NT):
            xr[:, it * C:(it + 1) * C] = xm[c * N + it * 128:c * N + (it + 1) * 128, :]
        m = dict(common)
        m["acore"] = np.ascontiguousarray(acore)
        m["xres"] = xr
        in_maps.append(m)
    return in_maps


def _get_nc():
    if "nc" not in _CACHE:
        _CACHE["nc"] = _build()
    return _CACHE["nc"]


def run_spmd(inputs, **kw):
    nc = _get_nc()
    in_maps = _host_inputs(inputs["x"], inputs["W_conv"], inputs["b_conv"],
                           inputs["gamma"], inputs["beta"])
    return bass_utils.run_bass_kernel_spmd(nc, in_maps, core_ids=list(range(NCORE)), **kw)


def kernel(**inputs):
    res = run_spmd(inputs)
    out = np.stack([res.results[c]["out"] for c in range(NCORE)], 0)
    return out.reshape(B, N, C).astype(np.float32)


# revision 52
# speedup vs baseline: 1.0593x; 1.0593x over previous
"""Trainium2 Bass kernel for nn_G_HGNN_layer_38448547234609.

HGNN layer: knn-hypergraph construction (top-11 nearest of 8192 nodes) +
static local-window hyperedges, G = Dv^-1/2 H De^-1 H^T Dv^-1/2 message
passing, linear -> G @ y -> batchnorm(train) -> relu -> residual.

Never materializes G: z = dv2 * (Hfull @ (1/DE * (Hfull^T @ (dv2*y)))).

Sharding: core c owns sample c (1024 nodes = 8 row-tiles of 128).
v2 design (vs baseline): knn masks kept SBUF-resident in fp8 (64KB/part,
no DRAM spill), u accumulated in two PSUM passes re-reading SBUF masks,
two pipelined AllReduce halves, masks transposed IN PLACE on PE (fp8,
batched 16-chunk PSUM evictions split Act/DVE) during the collectives,
z matmuls with fp8 lhsT x bf16 rhs.
"""

import numpy as np
import ml_dtypes

import concourse.bass as bass
import concourse.bacc as bacc
import concourse.mybir as mybir
import concourse.tile as tile
from concourse import bass_utils

AF = mybir.ActivationFunctionType
ALU = mybir.AluOpType
F32 = mybir.dt.float32
BF16 = mybir.dt.bfloat16
FP8 = mybir.dt.float8e4

NODE, K, KER, STR = 32, 10, 5, 2
B, C = 8, 64
N = NODE * NODE            # 1024 nodes/sample
BN = B * N                 # 8192
OUT_ = (NODE - KER) // STR + 1
E = OUT_ * OUT_            # 196 local hyperedges/sample
NCORE = 8
NT = 8                     # 128-row tiles per core
JC = 64                    # 128-col j-chunks
BN_EPS = 1e-5
BIG = 1e30

_CACHE = {}
SIM_NO_CC = False  # replace collectives with DMA copies (for TimelineSim)
DEBUG = False      # add intermediate dumps (dbg_* outputs)


def _local_incidence():
    idx = np.arange(N).reshape(NODE, NODE)
    H_local = np.zeros((N, E), np.float32)
    e = 0
    for i in range(0, NODE - KER + 1, STR):
        for j in range(0, NODE - KER + 1, STR):
            H_local[idx[i:i + KER, j:j + KER].ravel(), e] = 1.0
            e += 1
    return H_local


def _u_off(slot):
    # 32 PSUM accumulators of width 65 packed 7-per-512-f32-bank (5 banks)
    return (slot // 7) * 512 + (slot % 7) * 65


def _u_sb_off(jc):
    # u_sb mirrors the PSUM bank packing so drains are 5 wide copies per half
    h = 0 if jc < 32 else 32 * 65
    s = jc % 32
    return h + (s // 7) * 455 + (s % 7) * 65


_BANKW = [455, 455, 455, 455, 260]  # 7,7,7,7,4 slots of 65


def _build():
    nc = bacc.Bacc(num_devices=NCORE)

    bz = nc.dram_tensor("bz", [65, BN], F32, kind="ExternalInput")
    acore = nc.dram_tensor("acore", [65, N], F32, kind="ExternalInput")
    wb = nc.dram_tensor("wb", [65, C], F32, kind="ExternalInput")
    dv2t = nc.dram_tensor("dv2t", [128, NT], F32, kind="ExternalInput")
    hloc = nc.dram_tensor("hloc", [128, NT * E], BF16, kind="ExternalInput")
    hloct = nc.dram_tensor("hloct", [98, 2 * NT * 128], BF16, kind="ExternalInput")
    ident = nc.dram_tensor("ident", [128, 128], FP8, kind="ExternalInput")
    gamma = nc.dram_tensor("gamma", [1, C], F32, kind="ExternalInput")
    beta = nc.dram_tensor("beta", [1, C], F32, kind="ExternalInput")
    xres = nc.dram_tensor("xres", [128, NT * C], F32, kind="ExternalInput")
    out = nc.dram_tensor("out", [N, C], F32, kind="ExternalOutput")
    if DEBUG:
        dbg_u = nc.dram_tensor("dbg_u", [128, JC * 65], BF16, kind="ExternalOutput")
        dbg_ur = nc.dram_tensor("dbg_ur", [128, JC * 65], BF16, kind="ExternalOutput")
        dbg_v = nc.dram_tensor("dbg_v", [128, JC * C], BF16, kind="ExternalOutput")
        dbg_z = nc.dram_tensor("dbg_z", [128, NT * C], F32, kind="ExternalOutput")
        dbg_m = nc.dram_tensor("dbg_m", [128, NT * BN], FP8, kind="ExternalOutput")
        dbg_st = nc.dram_tensor("dbg_st", [1, 128], F32, kind="ExternalOutput")
        dbg_mt = nc.dram_tensor("dbg_mt", [128, NT * BN], FP8, kind="ExternalOutput")

    with tile.TileContext(nc) as tc:
        with (
            tc.tile_pool(name="const", bufs=1) as cp,
            tc.tile_pool(name="small", bufs=4) as sp,
            tc.tile_pool(name="persist", bufs=1) as pp,
            tc.tile_pool(name="dram", bufs=1, space="DRAM") as dr,
        ):
            # ---- const loads: small tensors P0/P1 need first, bz split so
            # P1's first chunks start early, late-phase consts last ----
            ac_sb = cp.tile([65, N], F32, tag="ac")
            nc.sync.dma_start(ac_sb[:], acore[:])
            wb_sb = cp.tile([65, C], F32, tag="wb")
            nc.sync.dma_start(wb_sb[:], wb[:])
            dv2_sb = cp.tile([128, NT], F32, tag="dv2")
            nc.sync.dma_start(dv2_sb[:], dv2t[:])
            bz_sb = cp.tile([65, BN], F32, tag="bz")
            for q in range(4):
                nc.sync.dma_start(bz_sb[:, q * 2048:(q + 1) * 2048],
                                  bz[:, q * 2048:(q + 1) * 2048])
            hloc_sb = cp.tile([128, NT * E], BF16, tag="hloc")
            nc.sync.dma_start(hloc_sb[:], hloc[:])
            hloct_sb = cp.tile([98, 2 * NT * 128], BF16, tag="hloct")
            nc.sync.dma_start(hloct_sb[:], hloct[:])
            id_sb = cp.tile([128, 128], FP8, tag="ident")
            nc.sync.dma_start(id_sb[:], ident[:])
            gam_sb = cp.tile([1, C], F32, tag="gamma")
            nc.sync.dma_start(gam_sb[:], gamma[:])
            bet_sb = cp.tile([1, C], F32, tag="beta")
            nc.sync.dma_start(bet_sb[:], beta[:])
            xr_sb = cp.tile([128, NT * C], F32, tag="xres")
            nc.sync.dma_start(xr_sb[:], xres[:])

            ones_sb = pp.tile([128, 1], F32, tag="ones")
            nc.vector.memset(ones_sb[:], 1.0)
            m_aug = pp.tile([128, NT * 65], BF16, tag="maug")
            mask_sb = pp.tile([128, NT * BN], FP8, tag="mask")
            d_sb = pp.tile([128, BN], F32, tag="dsb")
            u_sb = pp.tile([128, JC * 65], BF16, tag="usb")
            ur_sb = pp.tile([128, JC * 65], BF16, tag="ursb")
            v_sb = pp.tile([128, JC * C], BF16, tag="vsb")
            vloc_sb = pp.tile([98, 2 * C], BF16, tag="vloc")
            z_sb = pp.tile([128, NT * C], F32, tag="zsb")
            zsq_sb = pp.tile([128, C], F32, tag="zsq")

            def slot(it, jc):
                return mask_sb[:, it * BN + jc * 128:it * BN + (jc + 1) * 128]

            # ---- P0: y = x W^T + b ; m = dv2*y (bf16), ones col ----
            nc.vector.memset(m_aug[:], 1.0)  # ones cols; m cols overwritten below
            # prefetch the Sqrt act table so BN's sqrt doesn't pay the load
            sq_warm = sp.tile([1, 1], F32, tag="sqw")
            nc.vector.memset(sq_warm[:], 1.0)
            nc.scalar.activation(sq_warm[:], sq_warm[:], AF.Sqrt, bias=0.0, scale=1.0)
            with tc.tile_pool(name="py", bufs=2, space="PSUM") as pyp:
                for it in range(NT):
                    y_ps = pyp.tile([128, C], F32, tag="y")
                    nc.tensor.matmul(y_ps[:], lhsT=ac_sb[:, it * 128:(it + 1) * 128],
                                     rhs=wb_sb[:], start=True, stop=True)
                    nc.scalar.activation(m_aug[:, it * 65:it * 65 + C], y_ps[:],
                                         AF.Copy, bias=0.0, scale=dv2_sb[:, it:it + 1])

            # ---- P1: distances, exact top-11 threshold, fp8 mask, u1 ----
            with (
                tc.tile_pool(name="pd", bufs=3, space="PSUM") as pdp,
                tc.tile_pool(name="pu", bufs=1, space="PSUM") as pup,
            ):
                u_ps = pup.tile([128, 5 * 512], F32, tag="u")
                nc.vector.memset(u_ps[:], 0.0)
                for it in range(NT):
                    for nck in range(16):
                        d_ps = pdp.tile([128, 512], F32, tag="dch")
                        nc.tensor.matmul(d_ps[:],
                                         lhsT=ac_sb[:, it * 128:(it + 1) * 128],
                                         rhs=bz_sb[:, nck * 512:(nck + 1) * 512],
                                         start=True, stop=True)
                        nc.scalar.copy(d_sb[:, nck * 512:(nck + 1) * 512], d_ps[:])
                    # top-8 per 512-wide segment -> 128 candidates/row.
                    # For this problem no row has >8 of its top-11 in one
                    # segment (max observed 6), so candidates contain the
                    # exact global top-11; T11 = 11th largest candidate.
                    cand = sp.tile([128, 128], F32, tag="cand")
                    for sg in range(16):
                        nc.vector.max(cand[:, sg * 8:(sg + 1) * 8],
                                      d_sb[:, sg * 512:(sg + 1) * 512])
                    c8a = sp.tile([128, 8], F32, tag="v8")
                    nc.vector.max(c8a[:], cand[:])
                    nc.vector.match_replace(cand[:], c8a[:], cand[:], -BIG)
                    c8b = sp.tile([128, 8], F32, tag="v8")
                    nc.vector.max(c8b[:], cand[:])
                    # mask in 4 quarters (finer WAR granularity on d_sb)
                    for q in range(4):
                        nc.vector.tensor_scalar(
                            mask_sb[:, it * BN + q * 2048:it * BN + (q + 1) * 2048],
                            d_sb[:, q * 2048:(q + 1) * 2048],
                            c8b[:, 2:3], None, ALU.is_ge)
                    for jc in range(32):
                        o = _u_off(jc)
                        nc.tensor.matmul(u_ps[:, o:o + 65],
                                         lhsT=slot(it, jc),
                                         rhs=m_aug[:, it * 65:(it + 1) * 65],
                                         start=False, stop=(it == NT - 1),
                                         skip_group_check=True)
                # drain u1: one wide copy per PSUM bank; ship u1 half right away
                cc_in1 = dr.tile([128, JC * 65], BF16, tag="ccin1")
                cc_out1 = dr.tile([128, JC * 65], BF16, tag="ccout1", addr_space="Shared")
                for k in range(5):
                    w = _BANKW[k]
                    nc.scalar.copy(u_sb[:, k * 455:k * 455 + w],
                                   u_ps[:, k * 512:k * 512 + w])
                nc.sync.dma_start(cc_in1[:, 0:2080], u_sb[:, 0:2080])

                # ---- P2: u2 from SBUF-resident masks (jc-major; start resets,
                # verified clean on HW for the consecutive-block pattern) ----
                for jc in range(32, 64):
                    o = _u_off(jc - 32)
                    for it in range(NT):
                        nc.tensor.matmul(u_ps[:, o:o + 65],
                                         lhsT=slot(it, jc),
                                         rhs=m_aug[:, it * 65:(it + 1) * 65],
                                         start=(it == 0), stop=(it == NT - 1),
                                         skip_group_check=True)
                for k in range(5):
                    w = _BANKW[k]
                    nc.scalar.copy(u_sb[:, 2080 + k * 455:2080 + k * 455 + w],
                                   u_ps[:, k * 512:k * 512 + w])
                if DEBUG:
                    nc.sync.dma_start(dbg_u[:], u_sb[:])
                    nc.sync.dma_start(dbg_m[:], mask_sb[:])
                # single bf16 AllReduce of u (1.06MB)
                nc.sync.dma_start(cc_in1[:, 2080:], u_sb[:, 2080:])
                if SIM_NO_CC:
                    nc.sync.dma_start(cc_out1[:], cc_in1[:])
                else:
                    nc.gpsimd.collective_compute(
                        "AllReduce", ALU.add, replica_groups=[list(range(NCORE))],
                        ins=[cc_in1.opt()], outs=[cc_out1.opt()])

            # ---- local hyperedge t (independent of collectives) ----
            with tc.tile_pool(name="ptl", bufs=2, space="PSUM") as ptlp:
                tl_ps = [ptlp.tile([98, C], F32, tag=f"tl{ec}", name=f"tl_ps{ec}")
                         for ec in range(2)]
                for it in range(NT):
                    for ec in range(2):
                        nc.tensor.matmul(tl_ps[ec][:],
                                         lhsT=hloc_sb[:, it * E + ec * 98:it * E + ec * 98 + 98],
                                         rhs=m_aug[:, it * 65:it * 65 + C],
                                         start=(it == 0), stop=(it == NT - 1))
                for ec in range(2):
                    nc.scalar.activation(vloc_sb[:, ec * C:(ec + 1) * C], tl_ps[ec][:],
                                         AF.Copy, bias=0.0, scale=1.0 / 25.0)

            # ---- in-place mask transposes (fill PE during collective) ----
            # Regular matmul vs identity (is_transpose instructions get
            # serialized against collectives by the tile framework, which
            # pushes the whole phase out of the collective window).
            # 32 groups of 16 chunks; f32 PSUM (4 banks/group, 2 bufs);
            # evictions all on Act (cross-engine splits deadlock the
            # counter-encoded buf-rotation waits).
            with tc.tile_pool(name="ptr", bufs=2, space="PSUM") as ptrp:
                groups = [(it, h) for h in range(4) for it in range(NT)]
                for gi, (it, h) in enumerate(groups):
                    t_ps = ptrp.tile([128, 2048], F32, tag="tp")
                    for k in range(16):
                        jc = h * 16 + k
                        nc.tensor.matmul(t_ps[:, k * 128:(k + 1) * 128],
                                         lhsT=slot(it, jc), rhs=id_sb[:],
                                         start=True, stop=True)
                    dst = mask_sb[:, it * BN + h * 2048:it * BN + (h + 1) * 2048]
                    nc.scalar.copy(dst, t_ps[:])

            # ---- P4: readback + v = t / DE ----
            nc.sync.dma_start(ur_sb[:], cc_out1[:])
            # strided reciprocals of just the DE lanes -> compact [128, 64]
            urec = pp.tile([128, JC], F32, tag="urec")
            for g in range(2):
                for k in range(5):
                    w = 7 if k < 4 else 4
                    base = 2080 * g + 455 * k
                    nc.vector.reciprocal(
                        urec[:, 32 * g + 7 * k:32 * g + 7 * k + w],
                        ur_sb[:, base + C:base + C + (w - 1) * 65 + 1:65])
            for jc in range(JC):
                o = _u_sb_off(jc)
                nc.vector.tensor_scalar(v_sb[:, jc * C:(jc + 1) * C],
                                        ur_sb[:, o:o + C],
                                        urec[:, jc:jc + 1], None, ALU.mult)

            if DEBUG:
                nc.sync.dma_start(dbg_ur[:], ur_sb[:])
                nc.sync.dma_start(dbg_v[:], v_sb[:])
                nc.sync.dma_start(dbg_mt[:], mask_sb[:])

            # ---- P5: z = H v + local (8 z accumulators in one PSUM bank) ----
            with (
                tc.tile_pool(name="pz", bufs=1, space="PSUM") as pzp,
                tc.tile_pool(name="pst", bufs=1, space="PSUM") as pstp,
            ):
                z_ps = pzp.tile([128, NT * C], F32, tag="z")
                st_ps = pstp.tile([1, 128], F32, tag="st")
                nc.vector.memset(z_ps[:], 0.0)
                nc.vector.memset(st_ps[:], 0.0)
                # local part first (needs only vloc)
                for it in range(NT):
                    for ec in range(2):
                        nc.tensor.matmul(z_ps[:, it * C:(it + 1) * C],
                                         lhsT=hloct_sb[:, (ec * NT + it) * 128:(ec * NT + it + 1) * 128],
                                         rhs=vloc_sb[:, ec * C:(ec + 1) * C],
                                         start=False, stop=False,
                                         skip_group_check=True)
                for it in range(NT):
                    for jc in range(JC):
                        nc.tensor.matmul(z_ps[:, it * C:(it + 1) * C],
                                         lhsT=slot(it, jc),
                                         rhs=v_sb[:, jc * C:(jc + 1) * C],
                                         start=False, stop=(jc == JC - 1),
                                         skip_group_check=True)
                for it in range(NT):
                    # z scaled by dv2 on copy out
                    nc.scalar.activation(z_sb[:, it * C:(it + 1) * C],
                                         z_ps[:, it * C:(it + 1) * C],
                                         AF.Copy, bias=0.0, scale=dv2_sb[:, it:it + 1])
                    nc.vector.tensor_tensor(zsq_sb[:], z_sb[:, it * C:(it + 1) * C],
                                            z_sb[:, it * C:(it + 1) * C], ALU.mult)
                    nc.tensor.matmul(st_ps[0:1, 0:C], lhsT=ones_sb[:, 0:1],
                                     rhs=z_sb[:, it * C:(it + 1) * C],
                                     start=False, stop=(it == NT - 1),
                                     skip_group_check=True)
                    nc.tensor.matmul(st_ps[0:1, C:2 * C], lhsT=ones_sb[:, 0:1],
                                     rhs=zsq_sb[:],
                                     start=False, stop=(it == NT - 1),
                                     skip_group_check=True)
                st_sb = sp.tile([1, 128], F32, tag="stsb")
                nc.scalar.copy(st_sb[:], st_ps[:])
            if DEBUG:
                nc.sync.dma_start(dbg_z[:], z_sb[:])
                nc.sync.dma_start(dbg_st[:], st_sb[:])

            # ---- P6: AllGather stats + local partition-sum, BN coefficients ----
            st_in = dr.tile([1, 128], F32, tag="stin")
            st_out = dr.tile([NCORE, 128], F32, tag="stout", addr_space="Shared")
            nc.sync.dma_start(st_in[:], st_sb[:])
            if SIM_NO_CC:
                for c_ in range(NCORE):
                    nc.sync.dma_start(st_out[c_:c_ + 1, :], st_in[:])
            else:
                nc.gpsimd.collective_compute(
                    "AllGather", ALU.bypass, replica_groups=[list(range(NCORE))],
                    ins=[st_in.opt()], outs=[st_out.opt()])
            stg8 = sp.tile([NCORE, 128], F32, tag="stg8")
            nc.sync.dma_start(stg8[:], st_out[:])
            stg = sp.tile([1, 128], F32, tag="stg")
            with tc.tile_pool(name="pss", bufs=1, space="PSUM") as pssp:
                ss_ps = pssp.tile([1, 128], F32, tag="ss")
                nc.tensor.matmul(ss_ps[:], lhsT=ones_sb[0:NCORE, 0:1], rhs=stg8[:],
                                 start=True, stop=True)
                nc.scalar.copy(stg[:], ss_ps[:])

            mu = sp.tile([1, C], F32, tag="mu")
            nc.vector.tensor_scalar(mu[:], stg[0:1, 0:C], 1.0 / BN, None, ALU.mult)
            ex2 = sp.tile([1, C], F32, tag="ex2")
            nc.vector.tensor_scalar(ex2[:], stg[0:1, C:2 * C], 1.0 / BN, None, ALU.mult)
            musq = sp.tile([1, C], F32, tag="musq")
            nc.vector.tensor_tensor(musq[:], mu[:], mu[:], ALU.mult)
            var = sp.tile([1, C], F32, tag="var")
            nc.vector.tensor_tensor(var[:], ex2[:], musq[:], ALU.subtract)
            eps_t = sp.tile([1, 1], F32, tag="eps")
            nc.vector.memset(eps_t[:], BN_EPS)
            sd = sp.tile([1, C], F32, tag="sd")
            nc.scalar.activation(sd[:], var[:], AF.Sqrt, bias=eps_t[0:1, 0:1], scale=1.0)
            inv = sp.tile([1, C], F32, tag="inv")
            nc.vector.reciprocal(inv[:], sd[:])
            srow = sp.tile([1, C], F32, tag="srow")
            nc.vector.tensor_tensor(srow[:], gam_sb[:], inv[:], ALU.mult)
            msr = sp.tile([1, C], F32, tag="msr")
            nc.vector.tensor_tensor(msr[:], mu[:], srow[:], ALU.mult)
            trow = sp.tile([1, C], F32, tag="trow")
            nc.vector.tensor_tensor(trow[:], bet_sb[:], msr[:], ALU.subtract)
            s_b = pp.tile([128, C], F32, tag="sb_b")
            nc.gpsimd.partition_broadcast(s_b[:], srow[:])
            t_b = pp.tile([128, C], F32, tag="tb_b")
            nc.gpsimd.partition_broadcast(t_b[:], trow[:])

            # ---- P7: out = relu(z*s + t) + x ; single batched out DMA ----
            ot_sb = pp.tile([128, NT * C], F32, tag="otsb")
            for it in range(NT):
                tmp = sp.tile([128, C], F32, tag="tmp")
                nc.vector.tensor_tensor(tmp[:], z_sb[:, it * C:(it + 1) * C], s_b[:], ALU.mult)
                nc.vector.tensor_tensor(tmp[:], tmp[:], t_b[:], ALU.add)
                nc.scalar.activation(tmp[:], tmp[:], AF.Relu, bias=0.0, scale=1.0)
                nc.vector.tensor_tensor(ot_sb[:, it * C:(it + 1) * C], tmp[:],
                                        xr_sb[:, it * C:(it + 1) * C], ALU.add)
            nc.sync.dma_start(out.rearrange("(t p) c -> p t c", t=NT, p=128),
                              ot_sb[:].rearrange("p (t c) -> p t c", t=NT, c=C))

    nc.compile()
    return nc


def _static_host():
    if "static" not in _CACHE:
        H_local = _local_incidence()
        cover = H_local.sum(1)
        dv2 = ((K + 1 + cover) ** -0.5).astype(np.float32)
        dv2t = dv2.reshape(NT, 128).T.copy()  # [128, NT]

        hloc = np.zeros((128, NT * E), np.float32)
        for it in range(NT):
            hloc[:, it * E:(it + 1) * E] = H_local[it * 128:(it + 1) * 128, :]
        hloct = np.zeros((98, 2 * NT * 128), np.float32)
        for ec in range(2):
            for it in range(NT):
                blk = H_local[it * 128:(it + 1) * 128, ec * 98:ec * 98 + 98].T
                hloct[:, (ec * NT + it) * 128:(ec * NT + it + 1) * 128] = blk
        bf = ml_dtypes.bfloat16
        f8 = np.dtype(mybir.dt.np(FP8))
        _CACHE["static"] = {
            "dv2t": dv2t,
            "hloc": hloc.astype(bf),
            "hloct": hloct.astype(bf),
            "ident": np.eye(128, dtype=np.float32).astype(f8),
        }
    return _CACHE["static"]


def _host_inputs(x, W_conv, b_conv, gamma, beta):
    xm = np.ascontiguousarray(x.reshape(BN, C).astype(np.float32))
    xT = np.ascontiguousarray(xm.T)
    sq = (xm * xm).sum(1).astype(np.float32)

    bz = np.concatenate([2.0 * xT, -sq[None, :]], 0).astype(np.float32)
    wbm = np.concatenate([W_conv.T.astype(np.float32), b_conv[None, :].astype(np.float32)], 0)

    common = {
        "bz": bz,
        "wb": wbm,
        "gamma": np.ascontiguousarray(gamma.astype(np.float32)[None, :]),
        "beta": np.ascontiguousarray(beta.astype(np.float32)[None, :]),
        **_static_host(),
    }
    in_maps = []
    for c in range(NCORE):
        acore = np.concatenate(
            [xT[:, c * N:(c + 1) * N], np.ones((1, N), np.float32)], 0)
        xr = np.zeros((128, NT * C), np.float32)
        for it in range(NT):
            xr[:, it * C:(it + 1) * C] = xm[c * N + it * 128:c * N + (it + 1) * 128, :]
        m = dict(common)
        m["acore"] = np.ascontiguousarray(acore)
        m["xres"] = xr
        in_maps.append(m)
    return in_maps


def _get_nc():
    if "nc" not in _CACHE:
        _CACHE["nc"] = _build()
    return _CACHE["nc"]


def run_spmd(inputs, **kw):
    nc = _get_nc()
    in_maps = _host_inputs(inputs["x"], inputs["W_conv"], inputs["b_conv"],
                           inputs["gamma"], inputs["beta"])
    return bass_utils.run_bass_kernel_spmd(nc, in_maps, core_ids=list(range(NCORE)), **kw)


def kernel(**inputs):
    res = run_spmd(inputs)
    out = np.stack([res.results[c]["out"] for c in range(NCORE)], 0)
    return out.reshape(B, N, C).astype(np.float32)
